# revision 1
# baseline (speedup 1.0000x reference)
"""Trainium2 Bass kernel for BaseLayerWithLoRA: out = x @ W.T + b + (x @ A.T) @ B.T.

Shapes (hardcoded): x (8,16,8192) f32, W (8192,8192) f32, b (8192,) f32,
lora_A (16,8192) f32, lora_B (8192,16) f32. Output (8,16,8192) f32.

Strategy: tensor-parallel over out_features (Dout=8192) across 8 cores,
1024 outputs per core; x / lora_A replicated. All matmul operands cast to
fp16 on host (PSUM accumulates fp32; measured rel err ~3e-4). Host
pre-transposes x, lora_A, W so every DMA is a contiguous partition-major
load; bias is folded into the LoRA matmul as a rank-1 term with a
constant-ones row.
"""

import sys

for p in ("/opt/trn_rl_repo",):
    if p not in sys.path:
        sys.path.insert(0, p)

import numpy as np

import concourse.bacc as bacc
import concourse.bass as bass
import concourse.mybir as mybir
import concourse.tile as tile
from concourse.bass_utils import run_bass_kernel_spmd


def _ensure_axon_hooks_stub():
    """run_bass_kernel_spmd imports antenv.axon_hooks when BASS_TRACE is set;
    this container's antenv stub lacks it. Register a no-op fallback so the
    trace path degrades gracefully instead of crashing."""
    try:
        import antenv.axon_hooks  # noqa: F401
    except ImportError:
        import types

        import antenv

        mod = types.ModuleType("antenv.axon_hooks")
        _hook = [None]
        mod.get_axon_ntff_profile_hook = lambda: _hook[0]
        mod.set_axon_ntff_profile_hook = lambda h: _hook.__setitem__(0, h)
        sys.modules["antenv.axon_hooks"] = mod
        antenv.axon_hooks = mod


_ensure_axon_hooks_stub()


def _trim_exit_barrier():
    """Drop the second all-engine barrier in TileContext's exit sequence.
    After drain + barrier, every engine's instruction stream simply ends; the
    gpsimd semaphore clears complete within its own stream, so the trailing
    barrier only adds ~1us to every kernel. Idempotent, process-local."""
    from concourse.vector_clock import ScopedClock

    if getattr(tile.TileContext, "_exit_barrier_trimmed", False):
        return

    def _drain_and_barrier(self, tick_clock, wait_clock):
        drain_inst = self.nc.sync.drain()
        wait_clock.add_sem_waits(
            drain_inst.ins, ScopedClock({None: tick_clock.global_clock})
        )
        self.nc.all_engine_barrier()
        popped = self.nc._tile_sem_poison_stack.pop()
        assert popped is self._sem_poison
        self.nc.clear_and_free_semaphores(list(self.sems.allocated().values()))

    tile.TileContext._drain_and_barrier = _drain_and_barrier
    tile.TileContext._exit_barrier_trimmed = True


_trim_exit_barrier()

# Problem constants
T = 128          # tokens = 8*16
DIN = 8192
DOUT = 8192
R = 16           # lora rank
NCORES = 8
DC = DOUT // NCORES      # 1024 out-features per core
KT = DIN // 128          # 64 k-tiles
KCHUNK = 4               # k-tiles per W DMA chunk
NCHUNK = KT // KCHUNK    # 16 W chunks per do-half (0.5 MiB each)
F16 = mybir.dt.float16
F32 = mybir.dt.float32

_CACHE = {}
LAST_RESULT = None


def build_bass():
    nc = bacc.Bacc("TRN2", target_bir_lowering=False)
    # at and xt fused into one tensor: axt[p, k, 0:R] = lora_A.T tile,
    # axt[p, k, R:R+T] = x.T tile — loads in a single DMA so the W stream's
    # descriptors issue as early as possible.
    axt_d = nc.dram_tensor("axt", [128, KT, R + T], F16, kind="ExternalInput")
    # W stream is do-half-major: all 64 k-tiles for do[0:512], then do[512:1024]
    wt_d = nc.dram_tensor(
        "wt", [2, NCHUNK, 128, KCHUNK * 512], F16, kind="ExternalInput"
    )
    bb_d = nc.dram_tensor("bb", [R + 1, DC], F16, kind="ExternalInput")
    out_d = nc.dram_tensor("out", [T, DC], F32, kind="ExternalOutput")

    with tile.TileContext(nc) as tc:
        with (
            tc.tile_pool(name="res", bufs=1) as res,
            tc.tile_pool(name="wts", bufs=20) as wts,
            tc.tile_pool(name="outs", bufs=2) as outs,
            tc.tile_pool(name="ps", bufs=1, space="PSUM") as ps,
        ):
            # All loads ride one HWDGE ring (nc.sync) in strict priority
            # order: fused at+xt first (one DMA), then the W stream; bb is
            # deferred into the stream (only needed at the end of half 0).
            axt_s = res.tile([128, KT, R + T], F16)
            nc.sync.dma_start(out=axt_s[:], in_=axt_d[:, :, :])
            bb_s = res.tile([R + 1, DC], F16)

            psums = [
                ps.tile([T, 512], F32, tag="p0", name="psum0"),
                ps.tile([T, 512], F32, tag="p1", name="psum1"),
            ]
            psum_xa = ps.tile([R, T], F32, tag="pxa")
            xa_aug = res.tile([R + 1, T], F16)
            nc.vector.memset(xa_aug[:, :], 1.0)

            # do-half-major stream: psums[0] (do 0:512) completes mid-kernel,
            # so its bias+lora matmul, PSUM copy and output DMA all overlap
            # the second half's W stream. The 64 xa matmuls are spread over
            # the first half (4 per chunk) so xa_aug is ready by then.
            for h in range(2):
                psum = psums[h]
                if h == 1:
                    # Accumulation is commutative: seed psum1 with the
                    # bias+lora term (xa_aug is ready mid-half-0) so the
                    # post-stream tail is only the PSUM copy + output DMA.
                    nc.tensor.matmul(
                        psum[:], xa_aug[:], bb_s[:, 512:1024],
                        start=True, stop=False, skip_group_check=True,
                    )
                for c in range(NCHUNK):
                    if h == 0 and c == 2:
                        nc.sync.dma_start(out=bb_s[:], in_=bb_d[:, :])
                    wt_t = wts.tile([128, KCHUNK * 512], F16, tag="wt")
                    nc.sync.dma_start(out=wt_t[:], in_=wt_d[h, c])
                    # xa matmuls first: they only need axt, so PE starts on
                    # them while the first W chunk is still in flight.
                    if h == 0:
                        for kx in range(c * KCHUNK, (c + 1) * KCHUNK):
                            nc.tensor.matmul(
                                psum_xa[:], axt_s[:, kx, 0:R],
                                axt_s[:, kx, R : R + T],
                                start=(kx == 0), stop=(kx == KT - 1),
                                skip_group_check=True,
                            )
                    for s in range(KCHUNK):
                        k = c * KCHUNK + s
                        nc.tensor.matmul(
                            psum[:], axt_s[:, k, R : R + T],
                            wt_t[:, s * 512 : (s + 1) * 512],
                            start=(h == 0 and k == 0),
                            stop=(h == 1 and k == KT - 1),
                            skip_group_check=True,
                        )
                if h == 0:
                    # xa_aug rows 0..15 = (x @ A.T).T cast to fp16, row 16
                    # stays all-ones (folds the bias add into the matmul).
                    nc.vector.tensor_copy(xa_aug[0:R, :], psum_xa[:])
                    nc.tensor.matmul(
                        psum[:], xa_aug[:], bb_s[:, 0:512],
                        start=False, stop=True, skip_group_check=True,
                    )
                for piece in range(2):
                    ps_sl = slice(piece * 256, (piece + 1) * 256)
                    o_sl = slice(h * 512 + piece * 256, h * 512 + (piece + 1) * 256)
                    ot = outs.tile([T, 256], F32, tag=f"ot{piece}")
                    nc.vector.tensor_copy(ot[:], psum[:, ps_sl])
                    # In the tail (h=1) the W stream is done, so the sync ring
                    # is free: issue the two pieces on different rings so
                    # their ~0.6us issue costs overlap. Mid-kernel (h=0) both
                    # stay on scalar to keep the sync ring pure W.
                    eng = nc.sync if (h == 1 and piece == 0) else nc.scalar
                    eng.dma_start(out=out_d[:, o_sl], in_=ot[:])

    nc.compile()
    return nc


def _prep_inputs(x, W, b, lora_A, lora_B):
    xf = np.asarray(x, dtype=np.float32).reshape(T, DIN)
    # axt[p, k, 0:R] = A[r, 128k+p]; axt[p, k, R:R+T] = x[t, 128k+p]
    axt = np.empty((128, KT, R + T), np.float16)
    axt[:, :, :R] = (
        np.asarray(lora_A, np.float32).reshape(R, KT, 128).transpose(2, 1, 0)
    )
    axt[:, :, R:] = xf.reshape(T, KT, 128).transpose(2, 1, 0)
    W16 = np.asarray(W, np.float32).astype(np.float16)
    B16 = np.asarray(lora_B, np.float32).astype(np.float16)
    b16 = np.asarray(b, np.float32).astype(np.float16)
    in_maps = []
    for i in range(NCORES):
        sl = slice(i * DC, (i + 1) * DC)
        # wt[h, c, p, s*512 + n] = W[DC*i + 512h + n, 128*(KCHUNK*c+s) + p]
        wt = np.ascontiguousarray(
            W16[sl, :].T.reshape(NCHUNK, KCHUNK, 128, 2, 512)
            .transpose(3, 0, 2, 1, 4)
            .reshape(2, NCHUNK, 128, KCHUNK * 512)
        )
        bb = np.empty((R + 1, DC), np.float16)
        bb[:R] = B16[sl, :].T
        bb[R] = b16[sl]
        in_maps.append({"axt": axt, "wt": wt, "bb": bb})
    return in_maps


def kernel(x, W, b, lora_A, lora_B):
    global LAST_RESULT
    if "nc" not in _CACHE:
        _CACHE["nc"] = build_bass()
    nc = _CACHE["nc"]
    in_maps = _prep_inputs(x, W, b, lora_A, lora_B)
    res = run_bass_kernel_spmd(nc, in_maps, core_ids=list(range(NCORES)))
    LAST_RESULT = res
    out = np.concatenate([res.results[i]["out"] for i in range(NCORES)], axis=1)
    return np.ascontiguousarray(out.reshape(8, 16, DOUT), dtype=np.float32)



# revision 2
# speedup vs baseline: 1.3526x; 1.3526x over previous
"""Trainium2 Bass kernel for BaseLayerWithLoRA: out = x @ W.T + b + (x @ A.T) @ B.T.

Shapes (hardcoded): x (8,16,8192) f32, W (8192,8192) f32, b (8192,) f32,
lora_A (16,8192) f32, lora_B (8192,16) f32. Output (8,16,8192) f32.

Strategy: tensor-parallel over out_features (Dout=8192) across 8 cores,
1024 outputs per core; x replicated. The LoRA update is merged on host
(W' = W + B @ A — exact) so the device runs a single dense GEMM + bias.
W' is quantized to float8_e3m4 (4 mantissa bits) with a power-of-2 scale
folded into x (exact), halving W DMA traffic vs fp16; x stays fp16 as the
stationary operand (mixed-dtype matmul). Measured pipeline rel err ~9.7e-3
vs the 2e-2 gate.

Per core the stream is k-interleaved across two PSUM banks (out columns
0:512 and 512:1024) so one pass over the 64 k-tiles finishes both banks:
per 2 matmul-groups the DMA delivers 1 xt piece + 2 W chunks (~1.64us)
vs ~1.7us of PE work, keeping the DMA ahead and the PE continuously busy
(it ramps to the 2.4 GHz p-state and stays there — the baseline's PE was
DMA-starved and ran at ~1.2 GHz). Bank0 leads by 3 groups so its PSUM
copies and output DMAs overlap bank1's tail. Bias is folded in as a K=1
ones-row matmul at the end of each bank's accumulation group. Warmup
matmuls on a zeroed tile cover the DMA front so the PE p-state ramp
completes before real work arrives.
"""

import sys

for p in ("/opt/trn_rl_repo",):
    if p not in sys.path:
        sys.path.insert(0, p)

import math

import ml_dtypes
import numpy as np

import concourse.bacc as bacc
import concourse.bass as bass
import concourse.mybir as mybir
import concourse.tile as tile
from concourse.bass_utils import run_bass_kernel_spmd


def _ensure_axon_hooks_stub():
    """run_bass_kernel_spmd imports antenv.axon_hooks when BASS_TRACE is set;
    this container's antenv stub lacks it. Register a no-op fallback so the
    trace path degrades gracefully instead of crashing."""
    try:
        import antenv.axon_hooks  # noqa: F401
    except ImportError:
        import types

        import antenv

        mod = types.ModuleType("antenv.axon_hooks")
        _hook = [None]
        mod.get_axon_ntff_profile_hook = lambda: _hook[0]
        mod.set_axon_ntff_profile_hook = lambda h: _hook.__setitem__(0, h)
        sys.modules["antenv.axon_hooks"] = mod
        antenv.axon_hooks = mod


_ensure_axon_hooks_stub()


def _trim_exit_barrier():
    """Drop the second all-engine barrier in TileContext's exit sequence.
    After drain + barrier, every engine's instruction stream simply ends; the
    gpsimd semaphore clears complete within its own stream, so the trailing
    barrier only adds ~1us to every kernel. Idempotent, process-local."""
    from concourse.vector_clock import ScopedClock

    if getattr(tile.TileContext, "_exit_barrier_trimmed", False):
        return

    def _drain_and_barrier(self, tick_clock, wait_clock):
        drain_inst = self.nc.sync.drain()
        wait_clock.add_sem_waits(
            drain_inst.ins, ScopedClock({None: tick_clock.global_clock})
        )
        self.nc.all_engine_barrier()
        popped = self.nc._tile_sem_poison_stack.pop()
        assert popped is self._sem_poison
        self.nc.clear_and_free_semaphores(list(self.sems.allocated().values()))

    tile.TileContext._drain_and_barrier = _drain_and_barrier
    tile.TileContext._exit_barrier_trimmed = True


_trim_exit_barrier()

# Problem constants
T = 128          # tokens = 8*16
DIN = 8192
DOUT = 8192
R = 16           # lora rank
NCORES = 8
DC = DOUT // NCORES      # 1024 out-features per core
KT = DIN // 128          # 64 k-tiles
KG = 4                   # k-tiles per W chunk / xt piece
G = KT // KG             # 16 groups
NWARM = 9                # warmup matmuls bridging the DMA front
LEAD = 3                 # bank0 group-slots of lead over bank1
F16 = mybir.dt.float16
F8 = mybir.dt.float8e3
F32 = mybir.dt.float32

_CACHE = {}
LAST_RESULT = None


def build_bass():
    nc = bacc.Bacc("TRN2", target_bir_lowering=False)
    # xt[p, k, t] = (x * 2^-s)[t, 128k+p] fp16 — stationary operand tiles
    xt_d = nc.dram_tensor("xt", [128, KT, T], F16, kind="ExternalInput")
    # wt[bank, g, p, s*512+n] = (W' * 2^s)[DC*i + 512*bank + n, 128*(KG*g+s)+p]
    wt_d = nc.dram_tensor("wt", [2, G, 128, KG * 512], F8, kind="ExternalInput")
    bo_d = nc.dram_tensor("bo", [1, DC], F16, kind="ExternalInput")
    out_d = nc.dram_tensor("out", [T, DC], F16, kind="ExternalOutput")

    with tile.TileContext(nc) as tc:
        with (
            tc.tile_pool(name="res", bufs=1) as res,
            tc.tile_pool(name="ps", bufs=1, space="PSUM") as ps,
        ):
            xt_s = res.tile([128, KT, T], F16)
            wt_s = res.tile([128, 2, G, KG * 512], F8)
            bo_s = res.tile([1, DC], F16)
            ones = res.tile([1, T], F16)
            warm = res.tile([128, 512], F16)
            outs = res.tile([T, DC], F16)
            psum = [
                ps.tile([T, 512], F32, tag="p0", name="psum0"),
                ps.tile([T, 512], F32, tag="p1", name="psum1"),
            ]
            pwarm = ps.tile([T, 512], F32, tag="pw", name="psumw")

            nc.vector.memset(warm[:, :], 0.0)
            nc.vector.memset(ones[:, :], 1.0)

            # --- DMA program -------------------------------------------------
            # bias on the scalar (Activation) ring; everything else streams on
            # the sync (SP) ring in the order the PE consumes it. Per 2 PE
            # group-slots the stream carries 1 xt piece + 2 W chunks.
            nc.scalar.dma_start(out=bo_s[:], in_=bo_d[:, :])

            def xt_piece(j):
                nc.sync.dma_start(
                    out=xt_s[:, KG * j : KG * (j + 1), :],
                    in_=xt_d[:, KG * j : KG * (j + 1), :],
                )

            def w_chunk(bank, j):
                nc.sync.dma_start(out=wt_s[:, bank, j, :], in_=wt_d[bank, j])

            for j in range(LEAD):
                xt_piece(j)
                w_chunk(0, j)
            for j in range(LEAD, G):
                xt_piece(j)
                w_chunk(0, j)
                w_chunk(1, j - LEAD)
            for j in range(G - LEAD, G):
                w_chunk(1, j)

            # --- PE program --------------------------------------------------
            # Warmups (no DMA dependency) keep the PE busy while the front of
            # the stream lands, completing the p-state ramp.
            for w in range(NWARM):
                nc.tensor.matmul(
                    pwarm[:], warm[:, 0:T], warm[:, :],
                    start=(w == 0), stop=(w == NWARM - 1),
                    skip_group_check=True,
                )

            def mm_group(bank, j):
                for s in range(KG):
                    k = KG * j + s
                    nc.tensor.matmul(
                        psum[bank][:], xt_s[:, k, :],
                        wt_s[:, bank, j, s * 512 : (s + 1) * 512],
                        start=(k == 0), stop=False, skip_group_check=True,
                    )

            def mm_bias(bank):
                # bias as a K=1 rank-1 term closes the accumulation group
                nc.tensor.matmul(
                    psum[bank][:], ones[:, :],
                    bo_s[:, bank * 512 : (bank + 1) * 512],
                    start=False, stop=True, skip_group_check=True,
                )

            # slot order: bank0 leads by LEAD groups, then alternate, then
            # bank1 drains — bank0's copies/output DMAs overlap bank1's tail.
            for j in range(LEAD):
                mm_group(0, j)
            for j in range(LEAD, G):
                mm_group(0, j)
                mm_group(1, j - LEAD)
            mm_bias(0)
            for piece in range(2):
                sl = slice(piece * 256, (piece + 1) * 256)
                nc.vector.tensor_copy(outs[:, sl], psum[0][:, sl])
                nc.scalar.dma_start(out=out_d[:, sl], in_=outs[:, sl])
            for j in range(G - LEAD, G):
                mm_group(1, j)
            mm_bias(1)
            for piece in range(2):
                sl = slice(512 + piece * 256, 512 + (piece + 1) * 256)
                ps_sl = slice(piece * 256, (piece + 1) * 256)
                nc.vector.tensor_copy(outs[:, sl], psum[1][:, ps_sl])
                eng = nc.sync if piece == 0 else nc.scalar
                eng.dma_start(out=out_d[:, sl], in_=outs[:, sl])

    nc.compile()
    return nc


def _prep_inputs(x, W, b, lora_A, lora_B):
    xf = np.asarray(x, dtype=np.float32).reshape(T, DIN)
    Wp = np.asarray(W, np.float32) + np.asarray(lora_B, np.float32) @ np.asarray(
        lora_A, np.float32
    )
    # largest power-of-2 scale keeping W' inside e3m4's finite range (+/-15.5)
    amax = float(np.abs(Wp).max())
    s = 2.0 ** math.floor(math.log2(15.0 / amax))
    W8 = np.clip(Wp * s, -15.5, 15.5).astype(ml_dtypes.float8_e3m4)
    # fold 1/s into x — exact (power-of-2 exponent shift in fp16)
    x16 = (xf.astype(np.float16)) * np.float16(1.0 / s)
    xt = np.ascontiguousarray(x16.reshape(T, KT, 128).transpose(2, 1, 0))
    b16 = np.asarray(b, np.float32).astype(np.float16)
    W8u = W8.view(np.uint8)
    in_maps = []
    for i in range(NCORES):
        sl = slice(i * DC, (i + 1) * DC)
        # wt[bank, g, p, s*512+n] = W8[DC*i + 512*bank + n, 128*(KG*g+s)+p]
        wt = np.ascontiguousarray(
            W8u[sl, :].T.reshape(G, KG, 128, 2, 512)
            .transpose(3, 0, 2, 1, 4)
            .reshape(2, G, 128, KG * 512)
        ).view(ml_dtypes.float8_e3m4)
        bo = np.ascontiguousarray(b16[sl].reshape(1, DC))
        in_maps.append({"xt": xt, "wt": wt, "bo": bo})
    return in_maps


def kernel(x, W, b, lora_A, lora_B):
    global LAST_RESULT
    if "nc" not in _CACHE:
        _CACHE["nc"] = build_bass()
    nc = _CACHE["nc"]
    in_maps = _prep_inputs(x, W, b, lora_A, lora_B)
    res = run_bass_kernel_spmd(nc, in_maps, core_ids=list(range(NCORES)))
    LAST_RESULT = res
    out = np.concatenate(
        [np.asarray(res.results[i]["out"]) for i in range(NCORES)], axis=1
    )
    return np.ascontiguousarray(out.reshape(8, 16, DOUT)).astype(np.float32)


# revision 5
# speedup vs baseline: 1.5305x; 1.1315x over previous
"""Trainium2 Bass kernel for BaseLayerWithLoRA: out = x @ W.T + b + (x @ A.T) @ B.T.

Shapes (hardcoded): x (8,16,8192) f32, W (8192,8192) f32, b (8192,) f32,
lora_A (16,8192) f32, lora_B (8192,16) f32. Output (8,16,8192) f32.

Strategy: tensor-parallel over out_features (Dout=8192) across 8 cores,
1024 outputs per core; x replicated. The LoRA update is merged on host
(W' = W + B @ A — exact) so the device runs a single dense GEMM + bias.
W' is quantized to float8_e3m4 (4 mantissa bits) with a power-of-2 scale
folded into x (exact), halving W DMA traffic vs fp16; x stays fp16 as the
stationary operand (mixed-dtype matmul). Measured pipeline rel err ~9.7e-3
vs the 2e-2 gate.

Per core the stream is k-interleaved across two PSUM banks (out columns
0:512 and 512:1024) so one pass over the 64 k-tiles finishes both banks:
per 2 matmul-groups the DMA delivers 1 xt piece + 2 W chunks (~1.64us)
vs ~1.7us of PE work, keeping the DMA ahead and the PE continuously busy
(it ramps to the 2.4 GHz p-state and stays there — the baseline's PE was
DMA-starved and ran at ~1.2 GHz). Bank0 leads by 3 groups so its PSUM
copies and output DMAs overlap bank1's tail. Bias is folded in as a K=1
ones-row matmul at the end of each bank's accumulation group. Warmup
matmuls on a zeroed tile cover the DMA front so the PE p-state ramp
completes before real work arrives.
"""

import sys

for p in ("/opt/trn_rl_repo",):
    if p not in sys.path:
        sys.path.insert(0, p)

import math

import ml_dtypes
import numpy as np

import concourse.bacc as bacc
import concourse.bass as bass
import concourse.mybir as mybir
import concourse.tile as tile
from concourse.bass_utils import run_bass_kernel_spmd


def _ensure_axon_hooks_stub():
    """run_bass_kernel_spmd imports antenv.axon_hooks when BASS_TRACE is set;
    this container's antenv stub lacks it. Register a no-op fallback so the
    trace path degrades gracefully instead of crashing."""
    try:
        import antenv.axon_hooks  # noqa: F401
    except ImportError:
        import types

        import antenv

        mod = types.ModuleType("antenv.axon_hooks")
        _hook = [None]
        mod.get_axon_ntff_profile_hook = lambda: _hook[0]
        mod.set_axon_ntff_profile_hook = lambda h: _hook.__setitem__(0, h)
        sys.modules["antenv.axon_hooks"] = mod
        antenv.axon_hooks = mod


_ensure_axon_hooks_stub()


def _trim_exit_barrier():
    """Drop the second all-engine barrier in TileContext's exit sequence.
    After drain + barrier, every engine's instruction stream simply ends; the
    gpsimd semaphore clears complete within its own stream, so the trailing
    barrier only adds ~1us to every kernel. Idempotent, process-local."""
    from concourse.vector_clock import ScopedClock

    if getattr(tile.TileContext, "_exit_barrier_trimmed", False):
        return

    def _drain_and_barrier(self, tick_clock, wait_clock):
        drain_inst = self.nc.sync.drain()
        wait_clock.add_sem_waits(
            drain_inst.ins, ScopedClock({None: tick_clock.global_clock})
        )
        self.nc.all_engine_barrier()
        popped = self.nc._tile_sem_poison_stack.pop()
        assert popped is self._sem_poison
        self.nc.clear_and_free_semaphores(list(self.sems.allocated().values()))

    tile.TileContext._drain_and_barrier = _drain_and_barrier
    tile.TileContext._exit_barrier_trimmed = True


_trim_exit_barrier()

# Problem constants
T = 128          # tokens = 8*16
DIN = 8192
DOUT = 8192
R = 16           # lora rank
NCORES = 8
DC = DOUT // NCORES      # 1024 out-features per core
KT = DIN // 128          # 64 k-tiles
KG = 8                   # k-tiles per W chunk (0.52 MB — keeps DMA issue-rate off the critical path)
G = KT // KG             # 8 groups per bank
XP = (8, 8, 16, 16, 16)  # xt piece sizes in k-tiles (small first pieces land sooner)
NWARM = 9                # warmup matmuls bridging the DMA front
LEAD = 2                 # bank0 group-slots of lead over bank1
F16 = mybir.dt.float16
F8 = mybir.dt.float8e3
F32 = mybir.dt.float32

_CACHE = {}
LAST_RESULT = None


def build_bass():
    nc = bacc.Bacc("TRN2", target_bir_lowering=False)
    # xt[p, k, t] = (x * 2^-s)[t, 128k+p] fp16 — stationary operand tiles
    xt_d = nc.dram_tensor("xt", [128, KT, T], F16, kind="ExternalInput")
    # wt[bank, g, p, s*512+n] = (W' * 2^s)[DC*i + 512*bank + n, 128*(KG*g+s)+p]
    wt_d = nc.dram_tensor("wt", [2, G, 128, KG * 512], F8, kind="ExternalInput")
    bo_d = nc.dram_tensor("bo", [1, DC], F16, kind="ExternalInput")
    out_d = nc.dram_tensor("out", [T, DC], F16, kind="ExternalOutput")

    with tile.TileContext(nc) as tc:
        with (
            tc.tile_pool(name="res", bufs=1) as res,
            tc.tile_pool(name="ps", bufs=1, space="PSUM") as ps,
        ):
            xt_s = res.tile([128, KT, T], F16)
            wt_s = res.tile([128, 2, G, KG * 512], F8)
            bo_s = res.tile([1, DC], F16)
            ones = res.tile([1, T], F16)
            warm = res.tile([128, 512], F16)
            outs = res.tile([T, DC], F16)
            psum = [
                ps.tile([T, 512], F32, tag="p0", name="psum0"),
                ps.tile([T, 512], F32, tag="p1", name="psum1"),
            ]
            pwarm = ps.tile([T, 512], F32, tag="pw", name="psumw")

            nc.vector.memset(warm[:, :], 0.0)
            nc.vector.memset(ones[:, :], 1.0)

            # --- DMA program -------------------------------------------------
            # bias on the scalar (Activation) ring; everything else streams on
            # the sync (SP) ring in the order the PE consumes it. Per 2 PE
            # group-slots the stream carries 1 xt piece + 2 W chunks.
            nc.scalar.dma_start(out=bo_s[:], in_=bo_d[:, :])

            xoff = [0]
            for n in XP:
                xoff.append(xoff[-1] + n)

            def xt_piece(i):
                nc.sync.dma_start(
                    out=xt_s[:, xoff[i] : xoff[i + 1], :],
                    in_=xt_d[:, xoff[i] : xoff[i + 1], :],
                )

            def w_chunk(bank, j):
                nc.sync.dma_start(out=wt_s[:, bank, j, :], in_=wt_d[bank, j])

            # xt piece i must precede the p0 group that first reads it; the
            # 2-W-chunks-per-2-slots cadence (~1.4us each) tracks the PE's
            # ~1.75us/group consumption with growing slack.
            xt_piece(0)
            w_chunk(0, 0)
            xt_piece(1)
            w_chunk(0, 1)
            xt_piece(2)
            w_chunk(0, 2)
            w_chunk(1, 0)
            xt_piece(3)
            w_chunk(0, 3)
            w_chunk(1, 1)
            xt_piece(4)
            for j in range(4, G):
                w_chunk(0, j)
                w_chunk(1, j - LEAD)
            for j in range(G - LEAD, G):
                w_chunk(1, j)

            # --- PE program --------------------------------------------------
            # Warmups (no DMA dependency) keep the PE busy while the front of
            # the stream lands, completing the p-state ramp.
            for w in range(NWARM):
                nc.tensor.matmul(
                    pwarm[:], warm[:, 0:T], warm[:, :],
                    start=(w == 0), stop=(w == NWARM - 1),
                    skip_group_check=True,
                )

            def mm_bias(bank):
                # bias as a K=1 rank-1 term SEEDS the accumulation group
                # (commutative) so the tail ends on a plain W matmul
                nc.tensor.matmul(
                    psum[bank][:], ones[:, :],
                    bo_s[:, bank * 512 : (bank + 1) * 512],
                    start=True, stop=False, skip_group_check=True,
                )

            def mm_group(bank, j):
                for s in range(KG):
                    k = KG * j + s
                    nc.tensor.matmul(
                        psum[bank][:], xt_s[:, k, :],
                        wt_s[:, bank, j, s * 512 : (s + 1) * 512],
                        start=False, stop=(k == KT - 1),
                        skip_group_check=True,
                    )

            # slot order: bank0 leads by LEAD groups, then alternate, then
            # bank1 drains — bank0's copies/output DMAs overlap bank1's tail.
            mm_bias(0)
            mm_bias(1)
            for j in range(LEAD):
                mm_group(0, j)
            for j in range(LEAD, G):
                mm_group(0, j)
                mm_group(1, j - LEAD)
            for piece in range(2):
                sl = slice(piece * 256, (piece + 1) * 256)
                nc.vector.tensor_copy(outs[:, sl], psum[0][:, sl])
                nc.scalar.dma_start(out=out_d[:, sl], in_=outs[:, sl])
            for j in range(G - LEAD, G):
                mm_group(1, j)
            # last half: 384-col piece then a small 128-col piece so the
            # serial tail (copy + DMA issue + transfer) is as short as possible
            for sl, ps_sl, eng in (
                (slice(512, 896), slice(0, 384), nc.scalar),
                (slice(896, 1024), slice(384, 512), nc.sync),
            ):
                nc.vector.tensor_copy(outs[:, sl], psum[1][:, ps_sl])
                eng.dma_start(out=out_d[:, sl], in_=outs[:, sl])

    nc.compile()
    return nc


def _prep_inputs(x, W, b, lora_A, lora_B):
    xf = np.asarray(x, dtype=np.float32).reshape(T, DIN)
    Wp = np.asarray(W, np.float32) + np.asarray(lora_B, np.float32) @ np.asarray(
        lora_A, np.float32
    )
    # largest power-of-2 scale keeping W' inside e3m4's finite range (+/-15.5)
    amax = float(np.abs(Wp).max())
    s = 2.0 ** math.floor(math.log2(15.0 / amax))
    W8 = np.clip(Wp * s, -15.5, 15.5).astype(ml_dtypes.float8_e3m4)
    # fold 1/s into x — exact (power-of-2 exponent shift in fp16)
    x16 = (xf.astype(np.float16)) * np.float16(1.0 / s)
    xt = np.ascontiguousarray(x16.reshape(T, KT, 128).transpose(2, 1, 0))
    b16 = np.asarray(b, np.float32).astype(np.float16)
    W8u = W8.view(np.uint8)
    in_maps = []
    for i in range(NCORES):
        sl = slice(i * DC, (i + 1) * DC)
        # wt[bank, g, p, s*512+n] = W8[DC*i + 512*bank + n, 128*(KG*g+s)+p]
        wt = np.ascontiguousarray(
            W8u[sl, :].T.reshape(G, KG, 128, 2, 512)
            .transpose(3, 0, 2, 1, 4)
            .reshape(2, G, 128, KG * 512)
        ).view(ml_dtypes.float8_e3m4)
        bo = np.ascontiguousarray(b16[sl].reshape(1, DC))
        in_maps.append({"xt": xt, "wt": wt, "bo": bo})
    return in_maps


def kernel(x, W, b, lora_A, lora_B):
    global LAST_RESULT
    if "nc" not in _CACHE:
        _CACHE["nc"] = build_bass()
    nc = _CACHE["nc"]
    in_maps = _prep_inputs(x, W, b, lora_A, lora_B)
    res = run_bass_kernel_spmd(nc, in_maps, core_ids=list(range(NCORES)))
    LAST_RESULT = res
    out = np.concatenate(
        [np.asarray(res.results[i]["out"]) for i in range(NCORES)], axis=1
    )
    return np.ascontiguousarray(out.reshape(8, 16, DOUT)).astype(np.float32)
